# revision 26
# baseline (speedup 1.0000x reference)
"""Trainium2 Bass kernel for a fused multi-head attention block.

Reference computation (B=4, T=2048, D=1152, H=8, HD=144, full rotary):
    q,k,v = x@Wq.T, x@Wk.T, x@Wv.T   (per head)
    q,k   = rope(q, k, cos, sin)
    o     = softmax(q k^T / sqrt(HD)) v
    out   = o @ Wo.T
Sharding (8 cores): core c = (batch b = c//2, head-group hg = c%2).
Each core computes 4 heads of one batch and a partial output
out_part = o_local @ Wo[:, hg_cols].T ; host sums the two partials per batch.

Design notes (v2):
  * q/k are projected DIRECTLY into transposed layout qT/kT [head_dim, T]
    (weight chunk stationary, xT streaming) so no PE transposes are needed
    before the score matmuls.  v keeps the [T, head_dim] layout for PV.
  * rope in transposed layout: partner(d) = d+-72 is a PARTITION shift,
    done with SBUF->SBUF DMAs; cos/sin live in [dim, T] layout with the
    rotate-half sign folded into sin host-side.  3 tensor ops per block.
  * head_dim 144 = 128 (main block per head) + 16 (tail).  The 4 heads'
    tails are packed into one shared 128-row projection block (q rows
    0:64, k rows 64:128).  Score tail matmuls are ZERO-PADDED to K=128
    (kTBz[h]: only head h's 16 rows non-zero) so every matmul runs in the
    default 128x128 array mode -- tiling-mode switches drain the PE.
  * Scores are computed transposed (S^T [keys, q]) so PV needs no
    transpose; softmax denominator comes free via a ones column in v.
  * exp() without max-subtraction: |scores*scale| < ~6, safe in fp32.
  * Phase C: o^T via PE transposes, then out[t,e] accumulated K-outer so
    the oT stationaries' LDWEIGHTS hide under 3 matmuls each.
"""

import numpy as np

B, T, D, H = 4, 2048, 1152, 8
HL = 4              # heads per core
HD = 144            # head dim
DV = HL * HD        # 576, v/o width
NT = T // 128       # 16 t-tiles
KC = D // 128       # 9 contraction chunks
SCALE = float(HD) ** -0.5
NCORES = 8

_NC_CACHE = {}
GSZ = 4  # score key-tiles per burst group


def _build(debug=False, gsz=None):
    gsz = GSZ if gsz is None else gsz
    import concourse.bacc as bacc
    import concourse.mybir as mybir
    from concourse.tile import TileContext

    dt = mybir.dt
    f32, bf16 = dt.float32, dt.bfloat16
    AF = mybir.ActivationFunctionType

    nc = bacc.Bacc(
        "TRN2",
        target_bir_lowering=False,
        debug=debug,
        enable_asserts=False,
        num_devices=NCORES,
    )

    xT = nc.declare_dram_parameter("xT", [D, T], bf16, isOutput=False)
    wqM = nc.declare_dram_parameter("wqM", [D, 512], bf16, isOutput=False)
    wkM = nc.declare_dram_parameter("wkM", [D, 512], bf16, isOutput=False)
    wqkT = nc.declare_dram_parameter("wqkT", [D, 128], bf16, isOutput=False)
    wvT = nc.declare_dram_parameter("wvT", [D, DV], bf16, isOutput=False)
    woT = nc.declare_dram_parameter("woT", [DV, D], bf16, isOutput=False)
    cosmT = nc.declare_dram_parameter("cosmT", [128, T], bf16, isOutput=False)
    sinmT = nc.declare_dram_parameter("sinmT", [128, T], bf16, isOutput=False)
    costF = nc.declare_dram_parameter("costF", [128, T], bf16, isOutput=False)
    sintF = nc.declare_dram_parameter("sintF", [128, T], bf16, isOutput=False)
    identB = nc.declare_dram_parameter("identB", [128, 128], bf16, isOutput=False)
    out = nc.declare_dram_parameter("out", [T, D], f32, isOutput=True)

    with TileContext(nc) as tc:
        with tc.tile_pool(name="persist", bufs=1) as P0:
            ident_bf = P0.tile([128, 128], bf16, name="ident_bf", tag="ident_bf")
            nc.sync.dma_start(ident_bf[:], identB[:])

            qTa = [
                P0.tile([128, T], bf16, name=f"qTa{h}", tag=f"qTa{h}")
                for h in range(HL)
            ]
            kTa = [
                P0.tile([128, T], bf16, name=f"kTa{h}", tag=f"kTa{h}")
                for h in range(HL)
            ]
            # roped tails: rows 0:64 q (16h+j = head h dim 128+j),
            # rows 64:128 k
            qkTB = P0.tile([128, T], bf16, name="qkTB", tag="qkTB")
            # zero-padded per-head k-tail stationaries (rows 16h:16h+16)
            kTBz = [
                P0.tile([128, T], bf16, name=f"kTBz{h}", tag=f"kTBz{h}")
                for h in range(HL)
            ]
            vt = [
                P0.tile([128, HL * (HD + 1)], bf16, name=f"v{t}", tag=f"v{t}")
                for t in range(NT)
            ]

            # ---------------- Phase A: projections + rope ------------------
            with tc.tile_pool(name="pa", bufs=1) as pa:
                xAll = pa.tile([128, KC * T], bf16, name="xAll", tag="xAll")
                xtiles = [xAll[:, k * T : (k + 1) * T] for k in range(KC)]
                cosm_sb = pa.tile([128, T], bf16, name="cosm", tag="cosm")
                sinm_sb = pa.tile([128, T], bf16, name="sinm", tag="sinm")
                cost_sb = pa.tile([128, T], bf16, name="cost", tag="cost")
                sint_sb = pa.tile([128, T], bf16, name="sint", tag="sint")

                # ---- q/k transposed projections (weight stationary), then
                # ---- V last so its rope-independent matmuls keep the PE
                # ---- busy while the tail rope + kTBz scatter complete.
                with (
                    tc.tile_pool(name="paq", bufs=1) as paq,
                    tc.tile_pool(name="paqps", bufs=1, space="PSUM") as paqps,
                ):
                    # single-descriptor bulk loads: each dma_start costs
                    # ~0.6us of Sync-sequencer issue time, so batch the
                    # weight matrices into one DMA each and x per-chunk
                    wqkAll = paq.tile([128, KC * 128], bf16, name="wqkA", tag="wqkA")
                    wqk_t = [wqkAll[:, k * 128 : (k + 1) * 128] for k in range(KC)]
                    wmK = paq.tile([128, KC * 512], bf16, name="wmK", tag="wmK")
                    wmQ = paq.tile([128, KC * 512], bf16, name="wmQ", tag="wmQ")
                    nc.sync.dma_start(
                        wqkAll.rearrange("p (k c) -> p k c", k=KC),
                        wqkT.rearrange("(k p) c -> p k c", p=128),
                    )
                    # leading edge first: the prologue's k=0 matmuls wait
                    # only on these small transfers (region-level deps)
                    nc.sync.dma_start(wmK[:, 0:512], wkM[0:128, :])
                    nc.sync.dma_start(xtiles[0][:], xT[0:128, :])
                    nc.sync.dma_start(
                        wmK[:, 512:].rearrange("p (k c) -> p k c", k=KC - 1),
                        wkM[128:].rearrange("(k p) c -> p k c", p=128),
                    )
                    for k in range(1, KC):
                        # per-chunk so arrival granularity matches the
                        # prologue's ~1.7us/chunk consumption rate
                        nc.sync.dma_start(
                            xtiles[k][:], xT[k * 128 : (k + 1) * 128, :]
                        )
                    nc.sync.dma_start(
                        wmQ.rearrange("p (k c) -> p k c", k=KC),
                        wqM.rearrange("(k p) c -> p k c", p=128),
                    )
                    wm_t = [wmK[:, k * 512 : (k + 1) * 512] for k in range(KC)]
                    wvAll = pa.tile([128, KC * DV], bf16, name="wvA", tag="wvA")
                    wv_t = [wvAll[:, k * DV : (k + 1) * DV] for k in range(KC)]
                    nc.sync.dma_start(
                        wvAll.rearrange("p (k c) -> p k c", k=KC),
                        wvT.rearrange("(k p) c -> p k c", p=128),
                    )

                    for hh in range(HL):
                        nc.any.memset(kTBz[hh][:], 0.0)
                    nc.sync.dma_start(cosm_sb[:], cosmT[:])
                    nc.sync.dma_start(sinm_sb[:], sinmT[:])
                    nc.sync.dma_start(cost_sb[:], costF[:])
                    nc.sync.dma_start(sint_sb[:], sintF[:])

                    def block_mm(stat_fn):
                        ps = paqps.tile(
                            [128, T], f32, name="psQ", tag="qkps", bufs=2
                        )
                        for k in range(KC):
                            st, sp = k == 0, k == KC - 1
                            stat = stat_fn(k)
                            for c4 in range(4):
                                nc.tensor.matmul(
                                    ps[:, c4 * 512 : (c4 + 1) * 512],
                                    stat,
                                    xtiles[k][:, c4 * 512 : (c4 + 1) * 512],
                                    start=st,
                                    stop=sp,
                                )
                        return ps

                    def evac(ps, dst):
                        # per-bank copies (cross-bank PSUM reads are slow),
                        # split across both engines to halve evac latency
                        for c4 in range(4):
                            d = dst[:, c4 * 512 : (c4 + 1) * 512]
                            s = ps[:, c4 * 512 : (c4 + 1) * 512]
                            if c4 % 2 == 0:
                                nc.vector.tensor_copy(d, s)
                            else:
                                nc.scalar.copy(d, s)

                    tailraw = pa.tile([128, T], bf16, name="tailraw", tag="tailraw")
                    tailsh = pa.tile([128, T], bf16, name="tailsh", tag="tailsh")
                    tm1 = pa.tile([128, T], bf16, name="tm1", tag="tm1")

                    def finish_main(ps, dst_list, tail_part, h):
                        # tail_part: 0 for q (tailraw rows 0:64), 1 for k
                        raw = pa.tile(
                            [128, T], bf16, name="raw", tag="raw", bufs=2
                        )
                        evac(ps, raw)
                        sh = pa.tile(
                            [128, T], bf16, name="sh", tag="sh", bufs=2
                        )
                        tb = 64 * tail_part + 16 * h
                        nc.gpsimd.dma_start(sh[0:56, :], raw[72:128, :])
                        nc.gpsimd.dma_start(sh[56:72, :], tailraw[tb : tb + 16, :])
                        nc.gpsimd.dma_start(sh[72:128, :], raw[0:56, :])
                        # stash rows 56:72 (partner of the tail dims)
                        nc.gpsimd.dma_start(
                            tailsh[tb : tb + 16, :], raw[56:72, :]
                        )
                        m1 = pa.tile([128, T], bf16, name="m1", tag="m1", bufs=1)
                        m2 = pa.tile([128, T], bf16, name="m2", tag="m2", bufs=1)
                        nc.vector.tensor_mul(m1[:], raw[:], cosm_sb[:])
                        nc.vector.tensor_mul(m2[:], sh[:], sinm_sb[:])
                        nc.vector.tensor_add(dst_list[h][:], m1[:], m2[:])

                    def tail_half(tail_part):
                        # rope this half of the tail block (k half unblocks
                        # the kTBz scatter long before the q mains finish);
                        # slices keep all operands at the same base partition
                        lo = 64 * tail_part
                        tm2 = pa.tile([128, T], bf16, name="tm2", tag="m2", bufs=1)
                        nc.vector.tensor_mul(
                            tm2[lo : lo + 64, :],
                            tailsh[lo : lo + 64, :],
                            sint_sb[lo : lo + 64, :],
                        )
                        nc.vector.tensor_add(
                            qkTB[lo : lo + 64, :],
                            tm1[lo : lo + 64, :],
                            tm2[lo : lo + 64, :],
                        )
                        if tail_part == 1:
                            for hh in range(HL):
                                nc.gpsimd.dma_start(
                                    kTBz[hh][16 * hh : 16 * hh + 16, :],
                                    qkTB[64 + 16 * hh : 64 + 16 * hh + 16, :],
                                )

                    # interleaved prologue: the tail block and k-main h=0
                    # stream x together, so the startup DMA bandwidth (x not
                    # yet resident) feeds two blocks' worth of matmuls
                    ps_t = paqps.tile([128, T], f32, name="psQ", tag="qkps", bufs=2)
                    ps_k0 = paqps.tile([128, T], f32, name="psQ", tag="qkps", bufs=2)
                    for k in range(KC):
                        st, sp = k == 0, k == KC - 1
                        for c4 in range(4):
                            sl = slice(c4 * 512, (c4 + 1) * 512)
                            nc.tensor.matmul(
                                ps_t[:, sl], wqk_t[k][:], xtiles[k][:, sl],
                                start=st, stop=sp,
                            )
                            nc.tensor.matmul(
                                ps_k0[:, sl], wm_t[k][:, 0:128], xtiles[k][:, sl],
                                start=st, stop=sp,
                            )
                    evac(ps_t, tailraw)
                    # tail cos-product is ready as soon as tailraw lands
                    nc.vector.tensor_mul(tm1[:], tailraw[:], cost_sb[:])
                    finish_main(ps_k0, kTa, 1, 0)
                    for h in range(1, HL):
                        ps = block_mm(
                            lambda k, h=h: wm_t[k][:, 128 * h : 128 * (h + 1)]
                        )
                        finish_main(ps, kTa, 1, h)
                    tail_half(1)
                    wm_t = [wmQ[:, k * 512 : (k + 1) * 512] for k in range(KC)]
                    for h in range(HL):
                        ps = block_mm(
                            lambda k, h=h: wm_t[k][:, 128 * h : 128 * (h + 1)]
                        )
                        finish_main(ps, qTa, 0, h)
                    tail_half(0)

                    # ---- V projection (x stationary, wv streaming); psV
                    # shares the qkps slots so there is no pool-transition
                    # stall between the q mains and the v matmuls
                    for t in range(NT):
                        psV = paqps.tile(
                            [128, DV], f32, name="psV", tag="qkps", bufs=2
                        )
                        for k in range(KC):
                            st, sp = k == 0, k == KC - 1
                            lhs = xtiles[k][:, t * 128 : (t + 1) * 128]
                            nc.tensor.matmul(
                                psV[:, 0:512], lhs, wv_t[k][:, 0:512],
                                start=st, stop=sp,
                            )
                            nc.tensor.matmul(
                                psV[:, 512:DV], lhs, wv_t[k][:, 512:DV],
                                start=st, stop=sp,
                            )
                        v3 = vt[t].rearrange("p (h e) -> p h e", h=HL)
                        nc.any.tensor_copy(
                            v3[:, :, 0:HD],
                            psV.rearrange("p (h e) -> p h e", h=HL),
                        )
                        nc.vector.memset(v3[:, :, HD : HD + 1], 1.0)

            # ---------------- Phase B: attention --------------------------
            with tc.tile_pool(name="pb", bufs=1) as pb:
                ot = [
                    pb.tile([128, DV], bf16, name=f"o{t}", tag=f"o{t}")
                    for t in range(NT)
                ]
                with tc.tile_pool(name="pbps", bufs=1, space="PSUM") as pbps:
                    for qb in range(4):
                        for h in range(HL):
                            # pack the 4 q-tile accumulators into 2 PSUM banks:
                            # 3*145 fp32 = 1740B fits one 2KB bank
                            o_ps3 = pbps.tile(
                                [128, 3 * (HD + 1)], f32, name="o_ps3", tag="o3", bufs=1
                            )
                            o_ps1 = pbps.tile(
                                [128, HD + 1], f32, name="o_ps1", tag="o1", bufs=1
                            )
                            o_ps = [
                                o_ps3[:, 0 : HD + 1],
                                o_ps3[:, HD + 1 : 2 * (HD + 1)],
                                o_ps3[:, 2 * (HD + 1) : 3 * (HD + 1)],
                                o_ps1[:],
                            ]

                            def s_exp_group(g):
                                # 4 key-tiles per group, paired into 2-bank
                                # PSUM tiles; all matmuls K=128 (tail via
                                # zero-padded kTBz) -> no mode switches.
                                sps2 = [
                                    pbps.tile(
                                        [128, 1024], f32, name="sps", tag="sc", bufs=3
                                    )
                                    for _ in range(gsz // 2)
                                ]
                                for j in range(gsz):
                                    kt = gsz * g + j
                                    dst = sps2[j // 2][
                                        :, (j % 2) * 512 : (j % 2) * 512 + 512
                                    ]
                                    nc.tensor.matmul(
                                        dst,
                                        kTa[h][:, kt * 128 : (kt + 1) * 128],
                                        qTa[h][:, qb * 512 : (qb + 1) * 512],
                                        start=True,
                                        stop=False,
                                    )
                                for j in range(gsz):
                                    kt = gsz * g + j
                                    dst = sps2[j // 2][
                                        :, (j % 2) * 512 : (j % 2) * 512 + 512
                                    ]
                                    nc.tensor.matmul(
                                        dst,
                                        kTBz[h][:, kt * 128 : (kt + 1) * 128],
                                        qkTB[:, qb * 512 : (qb + 1) * 512],
                                        start=False,
                                        stop=True,
                                    )
                                Es = []
                                for j2 in range(gsz // 2):
                                    E = pb.tile(
                                        [128, 1024], bf16, name="E", tag="E", bufs=4
                                    )
                                    nc.scalar.activation(
                                        E[:], sps2[j2][:], AF.Exp, scale=SCALE
                                    )
                                    Es.append(E[:, 0:512])
                                    Es.append(E[:, 512:1024])
                                return Es

                            def pv_group(g, Es):
                                for j in range(gsz):
                                    kt = gsz * g + j
                                    for qt in range(4):
                                        # start/stop are bank-granular: qt 0-2
                                        # share o_ps3's bank
                                        if qt < 3:
                                            st = kt == 0 and qt == 0
                                            sp = kt == NT - 1 and qt == 2
                                        else:
                                            st = kt == 0
                                            sp = kt == NT - 1
                                        nc.tensor.matmul(
                                            o_ps[qt][:],
                                            Es[j][:, qt * 128 : (qt + 1) * 128],
                                            vt[kt][:, (HD + 1) * h : (HD + 1) * (h + 1)],
                                            start=st,
                                            stop=sp,
                                        )

                            ngrp = NT // gsz
                            Eprev = s_exp_group(0)
                            for g in range(ngrp):
                                Enext = s_exp_group(g + 1) if g + 1 < ngrp else None
                                pv_group(g, Eprev)
                                Eprev = Enext
                            for qt in range(4):
                                t = qb * 4 + qt
                                r = pb.tile([128, 1], f32, name="r", tag="r", bufs=4)
                                nc.vector.reciprocal(r[:], o_ps[qt][:, HD : HD + 1])
                                nc.vector.tensor_scalar_mul(
                                    ot[t][:, HD * h : HD * (h + 1)],
                                    o_ps[qt][:, 0:HD],
                                    r[:],
                                )

                # ---------------- Phase C: o^T + final projection ----------
                oTa = [
                    pb.tile([128, T], bf16, name=f"oTa{j}", tag=f"oTa{j}")
                    for j in range(4)
                ]
                oTb = pb.tile([128, T], bf16, name="oTb", tag="oTb")
                nc.vector.memset(oTb[64:128, :], 0.0)
                woAll = pb.tile([128, 5 * D], bf16, name="woA", tag="woA")
                wo_tiles = [woAll[:, k * D : (k + 1) * D] for k in range(5)]
                nc.sync.dma_start(
                    woAll[:, 0 : 4 * D].rearrange("p (k c) -> p k c", k=4),
                    woT[0:512].rearrange("(k p) c -> p k c", p=128),
                )
                nc.sync.dma_start(wo_tiles[4][0:64, :], woT[512:576, :])
                nc.vector.memset(wo_tiles[4][64:128, :], 0.0)
                with tc.tile_pool(name="pcps", bufs=1, space="PSUM") as pcps:

                    def o_transp(t):
                        for j in range(4):
                            tp = pcps.tile(
                                [128, 128], bf16, name="tpo", tag="otp", bufs=2
                            )
                            nc.tensor.transpose(
                                tp[:],
                                ot[t][:, 128 * j : 128 * (j + 1)],
                                ident_bf[:],
                            )
                            nc.any.tensor_copy(
                                oTa[j][:, t * 128 : (t + 1) * 128], tp[:]
                            )
                        tpb = pcps.tile([64, 128], bf16, name="tpb", tag="otp", bufs=2)
                        nc.tensor.transpose(
                            tpb[:],
                            ot[t][:, 512:DV],
                            ident_bf[:],
                        )
                        nc.any.tensor_copy(
                            oTb[0:64, t * 128 : (t + 1) * 128], tpb[:]
                        )

                    def final(t):
                        # K-outer: each oT stationary's LDWEIGHTS hides
                        # under the previous chunk's 3 matmuls
                        fps3 = [
                            pcps.tile(
                                [128, 384], f32, name=f"fps{j3}", tag=f"f{j3}", bufs=2
                            )
                            for j3 in range(3)
                        ]
                        for k in range(5):
                            lhs = (
                                oTa[k][:, t * 128 : (t + 1) * 128]
                                if k < 4
                                else oTb[:, t * 128 : (t + 1) * 128]
                            )
                            for j3 in range(3):
                                nc.tensor.matmul(
                                    fps3[j3][:],
                                    lhs,
                                    wo_tiles[k][:, 384 * j3 : 384 * (j3 + 1)],
                                    start=(k == 0),
                                    stop=(k == 4),
                                )
                        for j3 in range(3):
                            fout = pb.tile(
                                [128, 384], f32, name="fout", tag="fout", bufs=4
                            )
                            nc.any.tensor_copy(fout[:], fps3[j3][:])
                            nc.sync.dma_start(
                                out[
                                    t * 128 : (t + 1) * 128,
                                    384 * j3 : 384 * (j3 + 1),
                                ],
                                fout[:],
                            )

                    o_transp(0)
                    for t in range(NT):
                        if t + 1 < NT:
                            o_transp(t + 1)
                        final(t)

    nc.compile()
    return nc


def get_nc(debug=False, gsz=None):
    key = (bool(debug), GSZ if gsz is None else gsz)
    if key not in _NC_CACHE:
        _NC_CACHE[key] = _build(debug, gsz)
    return _NC_CACHE[key]


def make_in_maps(x, cos, sin, Wq, Wk, Wv, Wo):
    import ml_dtypes

    bf = ml_dtypes.bfloat16
    x = np.asarray(x, np.float32)
    cos = np.asarray(cos, np.float32)
    sin = np.asarray(sin, np.float32)
    Wq, Wk, Wv, Wo = (np.asarray(w, np.float32) for w in (Wq, Wk, Wv, Wo))

    cosT = cos.T  # [144, T]
    sinT = sin.T
    sign = np.where(np.arange(128) < 72, -1.0, 1.0).astype(np.float32)
    cosmT = np.ascontiguousarray(cosT[0:128]).astype(bf)
    sinmT = np.ascontiguousarray(sinT[0:128] * sign[:, None]).astype(bf)
    tidx = 128 + (np.arange(128) % 16)
    costF = np.ascontiguousarray(cosT[tidx]).astype(bf)
    sintF = np.ascontiguousarray(sinT[tidx]).astype(bf)

    in_maps = []
    for c in range(NCORES):
        b, hg = divmod(c, 2)
        heads = [HL * hg + i for i in range(HL)]

        def main_w(W):
            sel = np.concatenate(
                [W[144 * g : 144 * g + 128] for g in heads], 0
            )  # [512, D]
            return np.ascontiguousarray(sel.T).astype(bf)

        qk_tail = np.zeros((128, D), np.float32)
        for i, g in enumerate(heads):
            qk_tail[16 * i : 16 * i + 16] = Wq[144 * g + 128 : 144 * g + 144]
            qk_tail[64 + 16 * i : 64 + 16 * i + 16] = Wk[144 * g + 128 : 144 * g + 144]

        wv_sel = np.concatenate([Wv[144 * g : 144 * g + 144] for g in heads], 0)
        wo_sel = np.concatenate([Wo[:, 144 * g : 144 * g + 144] for g in heads], 1)
        in_maps.append(
            {
                "xT": np.ascontiguousarray(x[b].T).astype(bf),
                "wqM": main_w(Wq),
                "wkM": main_w(Wk),
                "wqkT": np.ascontiguousarray(qk_tail.T).astype(bf),
                "wvT": np.ascontiguousarray(wv_sel.T).astype(bf),
                "woT": np.ascontiguousarray(wo_sel.T).astype(bf),
                "cosmT": cosmT,
                "sinmT": sinmT,
                "costF": costF,
                "sintF": sintF,
                "identB": np.eye(128, dtype=bf),
            }
        )
    return in_maps


def kernel(x, cos, sin, Wq, Wk, Wv, Wo, _trace=False, _trace_kwargs=None):
    from concourse.bass_utils import run_bass_kernel_spmd

    nc = get_nc()
    in_maps = make_in_maps(x, cos, sin, Wq, Wk, Wv, Wo)
    res = run_bass_kernel_spmd(
        nc,
        in_maps,
        list(range(NCORES)),
        trace=_trace,
        **(_trace_kwargs or {}),
    )
    parts = [res.results[c]["out"] for c in range(NCORES)]
    outb = np.stack([parts[2 * b] + parts[2 * b + 1] for b in range(B)])
    if _trace:
        kernel.last_results = res
    return outb.astype(np.float32)


# revision 27
# speedup vs baseline: 1.1852x; 1.1852x over previous
"""Trainium2 Bass kernel for a fused multi-head attention block.

Reference computation (B=4, T=2048, D=1152, H=8, HD=144, full rotary):
    q,k,v = x@Wq.T, x@Wk.T, x@Wv.T   (per head)
    q,k   = rope(q, k, cos, sin)
    o     = softmax(q k^T / sqrt(HD)) v
    out   = o @ Wo.T
Sharding (8 cores): core c = (batch b = c//2, head-group hg = c%2).
Each core computes 4 heads of one batch and a partial output
out_part = o_local @ Wo[:, hg_cols].T ; host sums the two partials per batch.

Design notes (v2):
  * q/k are projected DIRECTLY into transposed layout qT/kT [head_dim, T]
    (weight chunk stationary, xT streaming) so no PE transposes are needed
    before the score matmuls.  v keeps the [T, head_dim] layout for PV.
  * rope in transposed layout: partner(d) = d+-72 is a PARTITION shift,
    done with SBUF->SBUF DMAs; cos/sin live in [dim, T] layout with the
    rotate-half sign folded into sin host-side.  3 tensor ops per block.
  * head_dim 144 = 128 (main block per head) + 16 (tail).  The 4 heads'
    tails are packed into one shared 128-row projection block (q rows
    0:64, k rows 64:128).  Score tail matmuls are ZERO-PADDED to K=128
    (kTBz[h]: only head h's 16 rows non-zero) so every matmul runs in the
    default 128x128 array mode -- tiling-mode switches drain the PE.
  * Scores are computed transposed (S^T [keys, q]) so PV needs no
    transpose; softmax denominator comes free via a ones column in v.
  * exp() without max-subtraction: |scores*scale| < ~6, safe in fp32.
  * Phase C: o^T via PE transposes, then out[t,e] accumulated K-outer so
    the oT stationaries' LDWEIGHTS hide under 3 matmuls each.
"""

import numpy as np

B, T, D, H = 4, 2048, 1152, 8
HL = 4              # heads per core
HD = 144            # head dim
DV = HL * HD        # 576, v/o width
NT = T // 128       # 16 t-tiles
KC = D // 128       # 9 contraction chunks
SCALE = float(HD) ** -0.5
NCORES = 8

_NC_CACHE = {}
GSZ = 4  # score key-tiles per burst group


def _build(debug=False, gsz=None):
    gsz = GSZ if gsz is None else gsz
    import concourse.bacc as bacc
    import concourse.mybir as mybir
    from concourse.tile import TileContext

    dt = mybir.dt
    f32, bf16 = dt.float32, dt.bfloat16
    AF = mybir.ActivationFunctionType

    nc = bacc.Bacc(
        "TRN2",
        target_bir_lowering=False,
        debug=debug,
        enable_asserts=False,
        num_devices=NCORES,
    )

    xT = nc.declare_dram_parameter("xT", [D, T], bf16, isOutput=False)
    wqM = nc.declare_dram_parameter("wqM", [D, 512], bf16, isOutput=False)
    wkM = nc.declare_dram_parameter("wkM", [D, 512], bf16, isOutput=False)
    wqkT = nc.declare_dram_parameter("wqkT", [D, 128], bf16, isOutput=False)
    wvT = nc.declare_dram_parameter("wvT", [D, DV], bf16, isOutput=False)
    woT = nc.declare_dram_parameter("woT", [DV, D], bf16, isOutput=False)
    cosmT = nc.declare_dram_parameter("cosmT", [128, T], bf16, isOutput=False)
    sinmT = nc.declare_dram_parameter("sinmT", [128, T], bf16, isOutput=False)
    costF = nc.declare_dram_parameter("costF", [128, T], bf16, isOutput=False)
    sintF = nc.declare_dram_parameter("sintF", [128, T], bf16, isOutput=False)
    identB = nc.declare_dram_parameter("identB", [128, 128], bf16, isOutput=False)
    out = nc.declare_dram_parameter("out", [T, D], f32, isOutput=True)

    with TileContext(nc) as tc:
        with tc.tile_pool(name="persist", bufs=1) as P0:
            ident_bf = P0.tile([128, 128], bf16, name="ident_bf", tag="ident_bf")
            nc.sync.dma_start(ident_bf[:], identB[:])

            qTa = [
                P0.tile([128, T], bf16, name=f"qTa{h}", tag=f"qTa{h}")
                for h in range(HL)
            ]
            kTa = [
                P0.tile([128, T], bf16, name=f"kTa{h}", tag=f"kTa{h}")
                for h in range(HL)
            ]
            # roped tails: rows 0:64 q (16h+j = head h dim 128+j),
            # rows 64:128 k
            qkTB = P0.tile([128, T], bf16, name="qkTB", tag="qkTB")
            # zero-padded per-head k-tail stationaries (rows 16h:16h+16)
            kTBz = [
                P0.tile([128, T], bf16, name=f"kTBz{h}", tag=f"kTBz{h}")
                for h in range(HL)
            ]
            vt = [
                P0.tile([128, HL * (HD + 1)], bf16, name=f"v{t}", tag=f"v{t}")
                for t in range(NT)
            ]

            # ---------------- Phase A: projections + rope ------------------
            with tc.tile_pool(name="pa", bufs=1) as pa:
                xAll = pa.tile([128, KC * T], bf16, name="xAll", tag="xAll")
                xtiles = [xAll[:, k * T : (k + 1) * T] for k in range(KC)]
                cosm_sb = pa.tile([128, T], bf16, name="cosm", tag="cosm")
                sinm_sb = pa.tile([128, T], bf16, name="sinm", tag="sinm")
                cost_sb = pa.tile([128, T], bf16, name="cost", tag="cost")
                sint_sb = pa.tile([128, T], bf16, name="sint", tag="sint")

                # ---- q/k transposed projections (weight stationary), then
                # ---- V last so its rope-independent matmuls keep the PE
                # ---- busy while the tail rope + kTBz scatter complete.
                with (
                    tc.tile_pool(name="paq", bufs=1) as paq,
                    tc.tile_pool(name="paqps", bufs=1, space="PSUM") as paqps,
                ):
                    # single-descriptor bulk loads: each dma_start costs
                    # ~0.6us of Sync-sequencer issue time, so batch the
                    # weight matrices into one DMA each and x per-chunk
                    wqkAll = paq.tile([128, KC * 128], bf16, name="wqkA", tag="wqkA")
                    wqk_t = [wqkAll[:, k * 128 : (k + 1) * 128] for k in range(KC)]
                    wmK = paq.tile([128, KC * 512], bf16, name="wmK", tag="wmK")
                    wmQ = paq.tile([128, KC * 512], bf16, name="wmQ", tag="wmQ")
                    nc.sync.dma_start(
                        wqkAll.rearrange("p (k c) -> p k c", k=KC),
                        wqkT.rearrange("(k p) c -> p k c", p=128),
                    )
                    # leading edge first: the prologue's k=0 matmuls wait
                    # only on these small transfers (region-level deps)
                    nc.sync.dma_start(wmK[:, 0:512], wkM[0:128, :])
                    nc.sync.dma_start(xtiles[0][:], xT[0:128, :])
                    nc.sync.dma_start(
                        wmK[:, 512:].rearrange("p (k c) -> p k c", k=KC - 1),
                        wkM[128:].rearrange("(k p) c -> p k c", p=128),
                    )
                    for k in range(1, KC):
                        # per-chunk so arrival granularity matches the
                        # prologue's ~1.7us/chunk consumption rate
                        nc.sync.dma_start(
                            xtiles[k][:], xT[k * 128 : (k + 1) * 128, :]
                        )
                    nc.sync.dma_start(
                        wmQ.rearrange("p (k c) -> p k c", k=KC),
                        wqM.rearrange("(k p) c -> p k c", p=128),
                    )
                    wm_t = [wmK[:, k * 512 : (k + 1) * 512] for k in range(KC)]
                    wvAll = pa.tile([128, KC * DV], bf16, name="wvA", tag="wvA")
                    wv_t = [wvAll[:, k * DV : (k + 1) * DV] for k in range(KC)]
                    nc.sync.dma_start(
                        wvAll.rearrange("p (k c) -> p k c", k=KC),
                        wvT.rearrange("(k p) c -> p k c", p=128),
                    )

                    for hh in range(HL):
                        nc.any.memset(kTBz[hh][:], 0.0)
                    nc.sync.dma_start(cosm_sb[:], cosmT[:])
                    nc.sync.dma_start(sinm_sb[:], sinmT[:])
                    nc.sync.dma_start(cost_sb[:], costF[:])
                    nc.sync.dma_start(sint_sb[:], sintF[:])

                    def block_mm(stat_fn):
                        ps = paqps.tile(
                            [128, T], f32, name="psQ", tag="qkps", bufs=2
                        )
                        for k in range(KC):
                            st, sp = k == 0, k == KC - 1
                            stat = stat_fn(k)
                            for c4 in range(4):
                                nc.tensor.matmul(
                                    ps[:, c4 * 512 : (c4 + 1) * 512],
                                    stat,
                                    xtiles[k][:, c4 * 512 : (c4 + 1) * 512],
                                    start=st,
                                    stop=sp,
                                )
                        return ps

                    def evac(ps, dst):
                        # per-bank copies (cross-bank PSUM reads are slow),
                        # split across both engines to halve evac latency
                        for c4 in range(4):
                            d = dst[:, c4 * 512 : (c4 + 1) * 512]
                            s = ps[:, c4 * 512 : (c4 + 1) * 512]
                            if c4 % 2 == 0:
                                nc.vector.tensor_copy(d, s)
                            else:
                                nc.scalar.copy(d, s)

                    tailraw = pa.tile([128, T], bf16, name="tailraw", tag="tailraw")
                    tailsh = pa.tile([128, T], bf16, name="tailsh", tag="tailsh")
                    tm1 = pa.tile([128, T], bf16, name="tm1", tag="tm1")

                    def finish_main(ps, dst_list, tail_part, h):
                        # tail_part: 0 for q (tailraw rows 0:64), 1 for k
                        raw = pa.tile(
                            [128, T], bf16, name="raw", tag="raw", bufs=2
                        )
                        evac(ps, raw)
                        sh = pa.tile(
                            [128, T], bf16, name="sh", tag="sh", bufs=2
                        )
                        tb = 64 * tail_part + 16 * h
                        nc.gpsimd.dma_start(sh[0:56, :], raw[72:128, :])
                        nc.gpsimd.dma_start(sh[56:72, :], tailraw[tb : tb + 16, :])
                        nc.gpsimd.dma_start(sh[72:128, :], raw[0:56, :])
                        # stash rows 56:72 (partner of the tail dims)
                        nc.gpsimd.dma_start(
                            tailsh[tb : tb + 16, :], raw[56:72, :]
                        )
                        m1 = pa.tile([128, T], bf16, name="m1", tag="m1", bufs=1)
                        m2 = pa.tile([128, T], bf16, name="m2", tag="m2", bufs=1)
                        nc.vector.tensor_mul(m1[:], raw[:], cosm_sb[:])
                        nc.vector.tensor_mul(m2[:], sh[:], sinm_sb[:])
                        nc.vector.tensor_add(dst_list[h][:], m1[:], m2[:])

                    def tail_half(tail_part):
                        # rope this half of the tail block (k half unblocks
                        # the kTBz scatter long before the q mains finish);
                        # slices keep all operands at the same base partition
                        lo = 64 * tail_part
                        tm2 = pa.tile([128, T], bf16, name="tm2", tag="m2", bufs=1)
                        nc.vector.tensor_mul(
                            tm2[lo : lo + 64, :],
                            tailsh[lo : lo + 64, :],
                            sint_sb[lo : lo + 64, :],
                        )
                        nc.vector.tensor_add(
                            qkTB[lo : lo + 64, :],
                            tm1[lo : lo + 64, :],
                            tm2[lo : lo + 64, :],
                        )
                        if tail_part == 1:
                            for hh in range(HL):
                                nc.gpsimd.dma_start(
                                    kTBz[hh][16 * hh : 16 * hh + 16, :],
                                    qkTB[64 + 16 * hh : 64 + 16 * hh + 16, :],
                                )

                    # interleaved prologue: the tail block and k-main h=0
                    # stream x together, so the startup DMA bandwidth (x not
                    # yet resident) feeds two blocks' worth of matmuls
                    ps_t = paqps.tile([128, T], f32, name="psQ", tag="qkps", bufs=2)
                    ps_k0 = paqps.tile([128, T], f32, name="psQ", tag="qkps", bufs=2)
                    for k in range(KC):
                        st, sp = k == 0, k == KC - 1
                        for c4 in range(4):
                            sl = slice(c4 * 512, (c4 + 1) * 512)
                            nc.tensor.matmul(
                                ps_t[:, sl], wqk_t[k][:], xtiles[k][:, sl],
                                start=st, stop=sp,
                            )
                            nc.tensor.matmul(
                                ps_k0[:, sl], wm_t[k][:, 0:128], xtiles[k][:, sl],
                                start=st, stop=sp,
                            )
                    evac(ps_t, tailraw)
                    # tail cos-product is ready as soon as tailraw lands
                    nc.vector.tensor_mul(tm1[:], tailraw[:], cost_sb[:])
                    finish_main(ps_k0, kTa, 1, 0)
                    for h in range(1, HL):
                        ps = block_mm(
                            lambda k, h=h: wm_t[k][:, 128 * h : 128 * (h + 1)]
                        )
                        finish_main(ps, kTa, 1, h)
                    tail_half(1)
                    wm_t = [wmQ[:, k * 512 : (k + 1) * 512] for k in range(KC)]
                    for h in range(HL):
                        ps = block_mm(
                            lambda k, h=h: wm_t[k][:, 128 * h : 128 * (h + 1)]
                        )
                        finish_main(ps, qTa, 0, h)
                    tail_half(0)

                    # ---- V projection (x stationary, wv streaming); psV
                    # shares the qkps slots so there is no pool-transition
                    # stall between the q mains and the v matmuls
                    for t in range(NT):
                        psV = paqps.tile(
                            [128, DV], f32, name="psV", tag="qkps", bufs=2
                        )
                        for k in range(KC):
                            st, sp = k == 0, k == KC - 1
                            lhs = xtiles[k][:, t * 128 : (t + 1) * 128]
                            nc.tensor.matmul(
                                psV[:, 0:512], lhs, wv_t[k][:, 0:512],
                                start=st, stop=sp,
                            )
                            nc.tensor.matmul(
                                psV[:, 512:DV], lhs, wv_t[k][:, 512:DV],
                                start=st, stop=sp,
                            )
                        v3 = vt[t].rearrange("p (h e) -> p h e", h=HL)
                        nc.any.tensor_copy(
                            v3[:, :, 0:HD],
                            psV.rearrange("p (h e) -> p h e", h=HL),
                        )
                        nc.vector.memset(v3[:, :, HD : HD + 1], 1.0)

            # ---------------- Phase B: attention --------------------------
            with tc.tile_pool(name="pb", bufs=1) as pb:
                ot = [
                    pb.tile([128, DV], bf16, name=f"o{t}", tag=f"o{t}")
                    for t in range(NT)
                ]
                with tc.tile_pool(name="pbps", bufs=1, space="PSUM") as pbps:
                    for qb in range(4):
                        for h in range(HL):
                            # pack the 4 q-tile accumulators into 2 PSUM banks:
                            # 3*145 fp32 = 1740B fits one 2KB bank
                            o_ps3 = pbps.tile(
                                [128, 3 * (HD + 1)], f32, name="o_ps3", tag="o3", bufs=1
                            )
                            o_ps1 = pbps.tile(
                                [128, HD + 1], f32, name="o_ps1", tag="o1", bufs=1
                            )
                            o_ps = [
                                o_ps3[:, 0 : HD + 1],
                                o_ps3[:, HD + 1 : 2 * (HD + 1)],
                                o_ps3[:, 2 * (HD + 1) : 3 * (HD + 1)],
                                o_ps1[:],
                            ]

                            def s_exp_group(g):
                                # 4 key-tiles per group, paired into 2-bank
                                # PSUM tiles; all matmuls K=128 (tail via
                                # zero-padded kTBz) -> no mode switches.
                                sps2 = [
                                    pbps.tile(
                                        [128, 1024], f32, name="sps", tag="sc", bufs=3
                                    )
                                    for _ in range(gsz // 2)
                                ]
                                for j in range(gsz):
                                    kt = gsz * g + j
                                    dst = sps2[j // 2][
                                        :, (j % 2) * 512 : (j % 2) * 512 + 512
                                    ]
                                    nc.tensor.matmul(
                                        dst,
                                        kTa[h][:, kt * 128 : (kt + 1) * 128],
                                        qTa[h][:, qb * 512 : (qb + 1) * 512],
                                        start=True,
                                        stop=False,
                                    )
                                for j in range(gsz):
                                    kt = gsz * g + j
                                    dst = sps2[j // 2][
                                        :, (j % 2) * 512 : (j % 2) * 512 + 512
                                    ]
                                    nc.tensor.matmul(
                                        dst,
                                        kTBz[h][:, kt * 128 : (kt + 1) * 128],
                                        qkTB[:, qb * 512 : (qb + 1) * 512],
                                        start=False,
                                        stop=True,
                                    )
                                Es = []
                                for j2 in range(gsz // 2):
                                    E = pb.tile(
                                        [128, 1024], bf16, name="E", tag="E", bufs=4
                                    )
                                    nc.scalar.activation(
                                        E[:], sps2[j2][:], AF.Exp, scale=SCALE
                                    )
                                    Es.append(E[:, 0:512])
                                    Es.append(E[:, 512:1024])
                                return Es

                            def pv_group(g, Es):
                                for j in range(gsz):
                                    kt = gsz * g + j
                                    for qt in range(4):
                                        # start/stop are bank-granular: qt 0-2
                                        # share o_ps3's bank
                                        if qt < 3:
                                            st = kt == 0 and qt == 0
                                            sp = kt == NT - 1 and qt == 2
                                        else:
                                            st = kt == 0
                                            sp = kt == NT - 1
                                        nc.tensor.matmul(
                                            o_ps[qt][:],
                                            Es[j][:, qt * 128 : (qt + 1) * 128],
                                            vt[kt][:, (HD + 1) * h : (HD + 1) * (h + 1)],
                                            start=st,
                                            stop=sp,
                                        )

                            ngrp = NT // gsz
                            Eprev = s_exp_group(0)
                            for g in range(ngrp):
                                Enext = s_exp_group(g + 1) if g + 1 < ngrp else None
                                pv_group(g, Eprev)
                                Eprev = Enext
                            for qt in range(4):
                                t = qb * 4 + qt
                                r = pb.tile([128, 1], f32, name="r", tag="r", bufs=4)
                                nc.vector.reciprocal(r[:], o_ps[qt][:, HD : HD + 1])
                                nc.vector.tensor_scalar_mul(
                                    ot[t][:, HD * h : HD * (h + 1)],
                                    o_ps[qt][:, 0:HD],
                                    r[:],
                                )

                # ---------------- Phase C: o^T + final projection ----------
                oTa = [
                    pb.tile([128, T], bf16, name=f"oTa{j}", tag=f"oTa{j}")
                    for j in range(4)
                ]
                oTb = pb.tile([128, T], bf16, name="oTb", tag="oTb")
                nc.vector.memset(oTb[64:128, :], 0.0)
                woAll = pb.tile([128, 5 * D], bf16, name="woA", tag="woA")
                wo_tiles = [woAll[:, k * D : (k + 1) * D] for k in range(5)]
                nc.sync.dma_start(
                    woAll[:, 0 : 4 * D].rearrange("p (k c) -> p k c", k=4),
                    woT[0:512].rearrange("(k p) c -> p k c", p=128),
                )
                nc.sync.dma_start(wo_tiles[4][0:64, :], woT[512:576, :])
                nc.vector.memset(wo_tiles[4][64:128, :], 0.0)
                with tc.tile_pool(name="pcps", bufs=1, space="PSUM") as pcps:

                    def o_transp(t):
                        for j in range(4):
                            tp = pcps.tile(
                                [128, 128], bf16, name="tpo", tag="otp", bufs=2
                            )
                            nc.tensor.transpose(
                                tp[:],
                                ot[t][:, 128 * j : 128 * (j + 1)],
                                ident_bf[:],
                            )
                            nc.any.tensor_copy(
                                oTa[j][:, t * 128 : (t + 1) * 128], tp[:]
                            )
                        tpb = pcps.tile([64, 128], bf16, name="tpb", tag="otp", bufs=2)
                        nc.tensor.transpose(
                            tpb[:],
                            ot[t][:, 512:DV],
                            ident_bf[:],
                        )
                        nc.any.tensor_copy(
                            oTb[0:64, t * 128 : (t + 1) * 128], tpb[:]
                        )

                    def final(t):
                        # K-outer: each oT stationary's LDWEIGHTS hides
                        # under the previous chunk's 3 matmuls
                        fps3 = [
                            pcps.tile(
                                [128, 384], f32, name=f"fps{j3}", tag=f"f{j3}", bufs=2
                            )
                            for j3 in range(3)
                        ]
                        for k in range(5):
                            lhs = (
                                oTa[k][:, t * 128 : (t + 1) * 128]
                                if k < 4
                                else oTb[:, t * 128 : (t + 1) * 128]
                            )
                            for j3 in range(3):
                                nc.tensor.matmul(
                                    fps3[j3][:],
                                    lhs,
                                    wo_tiles[k][:, 384 * j3 : 384 * (j3 + 1)],
                                    start=(k == 0),
                                    stop=(k == 4),
                                )
                        fout = pb.tile(
                            [128, D], f32, name="fout", tag="fout", bufs=2
                        )
                        for j3 in range(3):
                            nc.any.tensor_copy(
                                fout[:, 384 * j3 : 384 * (j3 + 1)], fps3[j3][:]
                            )
                        nc.sync.dma_start(
                            out[t * 128 : (t + 1) * 128, :], fout[:]
                        )

                    o_transp(0)
                    for t in range(NT):
                        if t + 1 < NT:
                            o_transp(t + 1)
                        final(t)

    nc.compile()
    return nc


def get_nc(debug=False, gsz=None):
    key = (bool(debug), GSZ if gsz is None else gsz)
    if key not in _NC_CACHE:
        _NC_CACHE[key] = _build(debug, gsz)
    return _NC_CACHE[key]


def make_in_maps(x, cos, sin, Wq, Wk, Wv, Wo):
    import ml_dtypes

    bf = ml_dtypes.bfloat16
    x = np.asarray(x, np.float32)
    cos = np.asarray(cos, np.float32)
    sin = np.asarray(sin, np.float32)
    Wq, Wk, Wv, Wo = (np.asarray(w, np.float32) for w in (Wq, Wk, Wv, Wo))

    cosT = cos.T  # [144, T]
    sinT = sin.T
    sign = np.where(np.arange(128) < 72, -1.0, 1.0).astype(np.float32)
    cosmT = np.ascontiguousarray(cosT[0:128]).astype(bf)
    sinmT = np.ascontiguousarray(sinT[0:128] * sign[:, None]).astype(bf)
    tidx = 128 + (np.arange(128) % 16)
    costF = np.ascontiguousarray(cosT[tidx]).astype(bf)
    sintF = np.ascontiguousarray(sinT[tidx]).astype(bf)

    in_maps = []
    for c in range(NCORES):
        b, hg = divmod(c, 2)
        heads = [HL * hg + i for i in range(HL)]

        def main_w(W):
            sel = np.concatenate(
                [W[144 * g : 144 * g + 128] for g in heads], 0
            )  # [512, D]
            return np.ascontiguousarray(sel.T).astype(bf)

        qk_tail = np.zeros((128, D), np.float32)
        for i, g in enumerate(heads):
            qk_tail[16 * i : 16 * i + 16] = Wq[144 * g + 128 : 144 * g + 144]
            qk_tail[64 + 16 * i : 64 + 16 * i + 16] = Wk[144 * g + 128 : 144 * g + 144]

        wv_sel = np.concatenate([Wv[144 * g : 144 * g + 144] for g in heads], 0)
        wo_sel = np.concatenate([Wo[:, 144 * g : 144 * g + 144] for g in heads], 1)
        in_maps.append(
            {
                "xT": np.ascontiguousarray(x[b].T).astype(bf),
                "wqM": main_w(Wq),
                "wkM": main_w(Wk),
                "wqkT": np.ascontiguousarray(qk_tail.T).astype(bf),
                "wvT": np.ascontiguousarray(wv_sel.T).astype(bf),
                "woT": np.ascontiguousarray(wo_sel.T).astype(bf),
                "cosmT": cosmT,
                "sinmT": sinmT,
                "costF": costF,
                "sintF": sintF,
                "identB": np.eye(128, dtype=bf),
            }
        )
    return in_maps


def kernel(x, cos, sin, Wq, Wk, Wv, Wo, _trace=False, _trace_kwargs=None):
    from concourse.bass_utils import run_bass_kernel_spmd

    nc = get_nc()
    in_maps = make_in_maps(x, cos, sin, Wq, Wk, Wv, Wo)
    res = run_bass_kernel_spmd(
        nc,
        in_maps,
        list(range(NCORES)),
        trace=_trace,
        **(_trace_kwargs or {}),
    )
    parts = [res.results[c]["out"] for c in range(NCORES)]
    outb = np.stack([parts[2 * b] + parts[2 * b + 1] for b in range(B)])
    if _trace:
        kernel.last_results = res
    return outb.astype(np.float32)
